# revision 1
# baseline (speedup 1.0000x reference)
"""Trainium2 Bass kernel for nn_AttentionCellEncoder.

Contract: kernel(**inputs) takes FULL unsharded inputs (as produced by
setup_inputs) and returns the FULL [2048, 256] float32 output. Internally
shards cells across 8 NeuronCores (data-parallel over the cell dimension,
chunk_features table replicated), runs a Bass/Tile kernel via
run_bass_kernel_spmd, and reassembles the output.

Self-contained: all shapes/sharding hardcoded.
"""

import numpy as np

import concourse.bass as bass
import concourse.mybir as mybir
import concourse.tile as tile
from concourse import bacc
from concourse.bass_utils import run_bass_kernel_spmd
from concourse.masks import make_identity

FP = mybir.dt.float32
P = 128

# Problem dims
NUM_HEADS = 8
NUM_CHUNKS, INPUT_DIM = 50000, 768   # D = 768
HIDDEN_DIM, OUTPUT_DIM = 512, 256    # H = 512
NUM_CELLS, MAX_LEN = 2048, 64        # C, L
HEAD_DIM = HIDDEN_DIM // NUM_HEADS   # 64

N_CORES = 8
CELLS_PER_CORE = NUM_CELLS // N_CORES          # 256
TILES_PER_CORE = CELLS_PER_CORE // 2           # 128 tiles of 2 cells / 128 tokens
TILES_PER_BLOCK = 4                            # 512 tokens per block
BLOCKS = TILES_PER_CORE // TILES_PER_BLOCK     # 32
DCH = INPUT_DIM // P                           # 6 d-chunks
HCH = HIDDEN_DIM // P                          # 4 h-chunks
TOK_BLK = TILES_PER_BLOCK * P                  # 512
CELL_GROUPS = CELLS_PER_CORE // P              # 2 output groups of 128 cells


def build_kernel(with_v_bias: bool, debug_stage: str | None = None,
                 repeat: int = 1, att_bufs: int = 2, poolt_bufs: int = 2):
    """Trace and compile the per-core SPMD kernel. Returns the Bass object.

    debug_stage: one of None/"gather"/"qkv"/"swap"/"exp"/"ctx" to truncate the
    kernel after that stage (bisection aid; output is then garbage).
    """
    nc = bacc.Bacc(None)

    table = nc.dram_tensor("table", [NUM_CHUNKS, INPUT_DIM], FP, kind="ExternalInput")
    wq_t = nc.dram_tensor("wq_t", [INPUT_DIM, HIDDEN_DIM], FP, kind="ExternalInput")
    wk_t = nc.dram_tensor("wk_t", [INPUT_DIM, HIDDEN_DIM], FP, kind="ExternalInput")
    wv_t = nc.dram_tensor("wv_t", [INPUT_DIM, HIDDEN_DIM], FP, kind="ExternalInput")
    wf_t = nc.dram_tensor("wf_t", [HIDDEN_DIM, OUTPUT_DIM], FP, kind="ExternalInput")
    bq_c = nc.dram_tensor("bq_c", [P, HCH], FP, kind="ExternalInput")
    bk_c = nc.dram_tensor("bk_c", [P, HCH], FP, kind="ExternalInput")
    bv_r = nc.dram_tensor("bv_r", [1, HIDDEN_DIM], FP, kind="ExternalInput")
    idx = nc.dram_tensor("idx", [CELLS_PER_CORE * MAX_LEN], mybir.dt.int32,
                         kind="ExternalInput")
    maskb = nc.dram_tensor("maskb", [CELLS_PER_CORE * MAX_LEN], FP,
                           kind="ExternalInput")
    u2 = nc.dram_tensor("u2", [TILES_PER_CORE * P, 2], FP, kind="ExternalInput")
    out = nc.dram_tensor("out", [CELLS_PER_CORE, OUTPUT_DIM], FP,
                         kind="ExternalOutput")

    STAGES = {None: 99, "gather": 0, "qkv": 1, "swap": 2, "v": 3, "exp": 4,
              "ctx": 5, "pool": 6}
    lvl = STAGES[debug_stage]
    dbg_tile = None

    with tile.TileContext(nc) as tc:
        with (
            tc.tile_pool(name="const", bufs=1) as cpool,
            tc.tile_pool(name="xp", bufs=3) as xpool,
            tc.tile_pool(name="blk", bufs=2) as bpool,
            tc.tile_pool(name="sm", bufs=3) as spool,
            tc.tile_pool(name="op", bufs=2) as opool,
            tc.tile_pool(name="ps", bufs=2, space="PSUM") as pspool,
        ):
            ident = cpool.tile([P, P], FP)
            make_identity(nc, ident[:])
            ones = cpool.tile([P, 1], FP)
            nc.gpsimd.memset(ones[:], 1.0)

            wq_sb = cpool.tile([P, DCH * HIDDEN_DIM], FP)
            wk_sb = cpool.tile([P, DCH * HIDDEN_DIM], FP)
            wv_sb = cpool.tile([P, DCH * HIDDEN_DIM], FP)
            for j in range(DCH):
                s = slice(j * HIDDEN_DIM, (j + 1) * HIDDEN_DIM)
                d = slice(j * P, (j + 1) * P)
                nc.sync.dma_start(out=wq_sb[:, s], in_=wq_t[d, :])
                nc.sync.dma_start(out=wk_sb[:, s], in_=wk_t[d, :])
                nc.sync.dma_start(out=wv_sb[:, s], in_=wv_t[d, :])
            wf_sb = cpool.tile([P, HCH * OUTPUT_DIM], FP)
            for c in range(HCH):
                nc.sync.dma_start(out=wf_sb[:, c * OUTPUT_DIM:(c + 1) * OUTPUT_DIM],
                                  in_=wf_t[c * P:(c + 1) * P, :])
            bq_sb = cpool.tile([P, HCH], FP)
            bk_sb = cpool.tile([P, HCH], FP)
            nc.sync.dma_start(out=bq_sb[:], in_=bq_c[:, :])
            nc.sync.dma_start(out=bk_sb[:], in_=bk_c[:, :])
            if with_v_bias:
                ones1 = cpool.tile([1, P], FP)
                nc.gpsimd.memset(ones1[:], 1.0)
                bv_sb = cpool.tile([1, HIDDEN_DIM], FP)
                nc.sync.dma_start(out=bv_sb[:], in_=bv_r[:, :])

            for rep in range(repeat):
                # pooledT columns accumulate here per group of 128 cells:
                # poolT[:, hc*128 + cell] = pooled_cell[hc*128:(hc+1)*128]
                poolT_ps = [None] * CELL_GROUPS

                for b in range(BLOCKS):
                    g = b // (BLOCKS // CELL_GROUPS)
                    if poolT_ps[g] is None:
                        poolT_ps[g] = pspool.tile([P, HIDDEN_DIM], FP, tag="poolT",
                                                  name=f"poolT{g}",
                                                  bufs=poolt_bufs)
                    # ---- gather + transpose: xT_blk[:, j*512 + tok] = x^T ----
                    xT = bpool.tile([P, DCH * TOK_BLK], FP, tag="xT")
                    for t in range(TILES_PER_BLOCK):
                        row0 = (b * TILES_PER_BLOCK + t) * P
                        idx_sb = spool.tile([P, 1], mybir.dt.int32, tag="idx")
                        nc.sync.dma_start(out=idx_sb[:, :1],
                                          in_=idx[row0:row0 + P, None])
                        x = xpool.tile([P, INPUT_DIM], FP, tag="x")
                        nc.gpsimd.indirect_dma_start(
                            out=x[:], out_offset=None, in_=table[:],
                            in_offset=bass.IndirectOffsetOnAxis(ap=idx_sb[:, :1], axis=0),
                        )
                        pa = pspool.tile([P, 512], FP, tag="xp")
                        for j in range(4):
                            nc.tensor.transpose(out=pa[:, j * P:(j + 1) * P],
                                                in_=x[:, j * P:(j + 1) * P],
                                                identity=ident[:])
                        nc.vector.tensor_copy(
                            out=xT[:].rearrange("p (j n) -> p j n", j=DCH)
                                [:, 0:4, t * P:(t + 1) * P],
                            in_=pa[:].rearrange("p (j n) -> p j n", j=4),
                        )
                        pb = pspool.tile([P, 512], FP, tag="xp")
                        for j in range(2):
                            nc.tensor.transpose(out=pb[:, j * P:(j + 1) * P],
                                                in_=x[:, (4 + j) * P:(5 + j) * P],
                                                identity=ident[:])
                        nc.vector.tensor_copy(
                            out=xT[:].rearrange("p (j n) -> p j n", j=DCH)
                                [:, 4:6, t * P:(t + 1) * P],
                            in_=pb[:, 0:2 * P].rearrange("p (j n) -> p j n", j=2),
                        )

                    if lvl < 1:
                        dbg_tile = xT
                        continue
                    # ---- qT, kT: weight-stationary, N=512 tokens ----
                    # qT layout: [128 part = 2 heads x 64 d, HCH chunks x 512 tok]
                    # *_sw = partition halves swapped (for diagonal-tile scores)
                    qT = bpool.tile([P, HCH * TOK_BLK], FP, tag="qT")
                    kT = bpool.tile([P, HCH * TOK_BLK], FP, tag="kT")
                    qT_sw = bpool.tile([P, HCH * TOK_BLK], FP, tag="qTsw")
                    kT_sw = bpool.tile([P, HCH * TOK_BLK], FP, tag="kTsw")
                    for (wsb, bsb, dst, dsw) in ((wq_sb, bq_sb, qT, qT_sw),
                                                 (wk_sb, bk_sb, kT, kT_sw)):
                        for hc in range(HCH):
                            acc = pspool.tile([P, TOK_BLK], FP, tag="acc")
                            for j in range(DCH):
                                nc.tensor.matmul(
                                    out=acc[:],
                                    lhsT=wsb[:, j * HIDDEN_DIM + hc * P:
                                             j * HIDDEN_DIM + (hc + 1) * P],
                                    rhs=xT[:, j * TOK_BLK:(j + 1) * TOK_BLK],
                                    start=(j == 0), stop=(j == DCH - 1),
                                )
                            nc.scalar.activation(
                                out=dst[:, hc * TOK_BLK:(hc + 1) * TOK_BLK],
                                in_=acc[:],
                                func=mybir.ActivationFunctionType.Identity,
                                bias=bsb[:, hc:hc + 1])
                        if lvl >= 2:
                            nc.sync.dma_start(out=dsw[0:64, :], in_=dst[64:P, :])
                            nc.sync.dma_start(out=dsw[64:P, :], in_=dst[0:64, :])

                    if lvl < 3:
                        dbg_tile = qT if lvl < 2 else qT_sw
                        continue
                    # ---- v: x-stationary per tile, [128 tok, 512 h] ----
                    v = bpool.tile([P, TILES_PER_BLOCK * HIDDEN_DIM], FP, tag="v")
                    for t in range(TILES_PER_BLOCK):
                        acc = pspool.tile([P, HIDDEN_DIM], FP, tag="acc")
                        nmm = DCH + (1 if with_v_bias else 0)
                        for j in range(DCH):
                            nc.tensor.matmul(
                                out=acc[:],
                                lhsT=xT[:, j * TOK_BLK + t * P:j * TOK_BLK + (t + 1) * P],
                                rhs=wv_sb[:, j * HIDDEN_DIM:(j + 1) * HIDDEN_DIM],
                                start=(j == 0), stop=(j == nmm - 1),
                            )
                        if with_v_bias:
                            nc.tensor.matmul(out=acc[:], lhsT=ones1[0:1, :],
                                             rhs=bv_sb[0:1, :], start=False, stop=True)
                        nc.vector.tensor_copy(
                            out=v[:, t * HIDDEN_DIM:(t + 1) * HIDDEN_DIM], in_=acc[:])

                    if lvl < 4:
                        dbg_tile = v
                        continue
                    # ---- attention per tile (2 cells) ----
                    for t in range(TILES_PER_BLOCK):
                        gt = b * TILES_PER_BLOCK + t      # global tile id
                        row0 = gt * P
                        mk = spool.tile([P, 1], FP, tag="mk")
                        nc.sync.dma_start(out=mk[:, :1], in_=maskb[row0:row0 + P, None])
                        u2_sb = spool.tile([P, 2], FP, tag="u2")
                        nc.sync.dma_start(out=u2_sb[:], in_=u2[row0:row0 + P, :])

                        # scores^T: [2c x 64 m, 8h x 64 l]; diagonal tiles only:
                        # head h data taken from the copy that has it at half c.
                        sc = pspool.tile([P, HIDDEN_DIM], FP, tag="att", bufs=att_bufs)
                        for h in range(NUM_HEADS):
                            hc = h // 2
                            for c in range(2):   # c inner: T0/T10 quads overlap
                                pr = slice(c * 64, c * 64 + 64)
                                kk, qq = (kT, qT) if h % 2 == c else (kT_sw, qT_sw)
                                fw = slice(hc * TOK_BLK + t * P + c * 64,
                                           hc * TOK_BLK + t * P + c * 64 + 64)
                                nc.tensor.matmul(
                                    out=sc[pr, h * 64:h * 64 + 64],
                                    lhsT=kk[pr, fw], rhs=qq[pr, fw],
                                    start=True, stop=True,
                                )
                        e = spool.tile([P, HIDDEN_DIM], FP, tag="e")
                        nc.scalar.activation(out=e[:], in_=sc[:],
                                             func=mybir.ActivationFunctionType.Exp,
                                             bias=mk[:, :1])

                        if lvl < 5:
                            dbg_tile = e
                            continue
                        # ctx (unnormalized) + per-(l,h) denominators
                        ctx = pspool.tile([P, HIDDEN_DIM], FP, tag="att", bufs=att_bufs)
                        sden = pspool.tile([P, NUM_HEADS], FP, tag="att", bufs=att_bufs)
                        for h in range(NUM_HEADS):
                            for c in range(2):   # c inner: T0/T10 quads overlap
                                el = e[c * 64:c * 64 + 64, h * 64:h * 64 + 64]
                                nc.tensor.matmul(
                                    out=ctx[c * 64:c * 64 + 64, h * 64:h * 64 + 64],
                                    lhsT=el,
                                    rhs=v[c * 64:c * 64 + 64,
                                          t * HIDDEN_DIM + h * 64:
                                          t * HIDDEN_DIM + h * 64 + 64],
                                    start=True, stop=True,
                                )
                                nc.tensor.matmul(
                                    out=sden[c * 64:c * 64 + 64, h:h + 1],
                                    lhsT=el, rhs=ones[c * 64:c * 64 + 64, 0:1],
                                    start=True, stop=True,
                                )
                        r = spool.tile([P, NUM_HEADS], FP, tag="r")
                        nc.vector.reciprocal(out=r[:], in_=sden[:])
                        cn = spool.tile([P, HIDDEN_DIM], FP, tag="cn")
                        nc.vector.tensor_tensor(
                            out=cn[:].rearrange("p (h d) -> p h d", h=NUM_HEADS),
                            in0=ctx[:].rearrange("p (h d) -> p h d", h=NUM_HEADS),
                            in1=r[:, :, None].to_broadcast([P, NUM_HEADS, HEAD_DIM]),
                            op=mybir.AluOpType.mult,
                        )
                        if lvl < 6:
                            dbg_tile = cn
                            continue
                        # pooled columns: poolT[:, hc*128 + cell_local] =
                        #   sum_l u2[l, c] * cn[l, hc*128:(hc+1)*128]
                        # (u2 col c is zero outside cell c's rows -> K=128, no tiling)
                        for c in range(2):
                            cell_local = gt * 2 + c - g * P
                            for hc in range(HCH):
                                nc.tensor.matmul(
                                    out=poolT_ps[g][:, hc * P + cell_local:
                                                    hc * P + cell_local + 1],
                                    lhsT=cn[:, hc * P:(hc + 1) * P],
                                    rhs=u2_sb[:, c:c + 1],
                                    start=True, stop=True,
                                )

                if lvl < 99:
                    if lvl >= 6:
                        for g in range(CELL_GROUPS):
                            pdbg = opool.tile([P, HIDDEN_DIM], FP, tag="pooledT",
                                              name=f"pdbg{g}")
                            nc.vector.tensor_copy(out=pdbg[:], in_=poolT_ps[g][:])
                            nc.sync.dma_start(out=out[0:P, :],
                                              in_=pdbg[:, 0:OUTPUT_DIM])
                    else:
                        nc.sync.dma_start(out=out[0:P, :],
                                          in_=dbg_tile[:, 0:OUTPUT_DIM])
                # ---- final projection per group of 128 cells ----
                for g in range(CELL_GROUPS if lvl >= 99 else 0):
                    pooledT = opool.tile([P, HIDDEN_DIM], FP, tag="pooledT")
                    nc.vector.tensor_copy(out=pooledT[:], in_=poolT_ps[g][:])
                    acc = pspool.tile([P, OUTPUT_DIM], FP, tag="acc")
                    for c in range(HCH):
                        nc.tensor.matmul(
                            out=acc[:], lhsT=pooledT[:, c * P:(c + 1) * P],
                            rhs=wf_sb[:, c * OUTPUT_DIM:(c + 1) * OUTPUT_DIM],
                            start=(c == 0), stop=(c == HCH - 1),
                        )
                    osb = opool.tile([P, OUTPUT_DIM], FP, tag="osb")
                    nc.scalar.activation(out=osb[:], in_=acc[:],
                                         func=mybir.ActivationFunctionType.Copy)
                    nc.sync.dma_start(out=out[g * P:(g + 1) * P, :], in_=osb[:])

    nc.compile()
    return nc


def preprocess(chunk_features, Wq, bq, Wk, bk, Wv, bv, W_in, b_in, Wo, bo,
               Wout, bout, cell_idx, cell_len):
    """Host-side weight folding + per-core input maps. Returns (in_maps, b_final,
    with_v_bias)."""
    f32 = np.float32
    cf = np.ascontiguousarray(np.asarray(chunk_features, f32))
    Wq, Wk, Wv = (np.asarray(w, f32) for w in (Wq, Wk, Wv))
    bq, bk, bv = (np.asarray(x, f32) for x in (bq, bk, bv))
    W_in = np.asarray(W_in, f32)
    b_in = np.asarray(b_in, f32)
    Wo, bo = np.asarray(Wo, f32), np.asarray(bo, f32)
    Wout, bout = np.asarray(Wout, f32), np.asarray(bout, f32)

    Wiq, Wik, Wiv = np.split(W_in, 3, axis=0)
    biq, bik, biv = np.split(b_in, 3)
    scale = f32(1.0 / np.sqrt(HEAD_DIM))
    wq_eff = (Wiq @ Wq) * scale          # [512, 768]
    wk_eff = Wik @ Wk
    wv_eff = Wiv @ Wv
    bq_eff = (Wiq @ bq + biq) * scale    # [512]
    bk_eff = Wik @ bk + bik
    bv_eff = Wiv @ bv + biv
    wfin = Wout @ Wo                     # [256, 512]
    b_final = bo @ Wout.T + bout         # [256]

    wq_t = np.ascontiguousarray(wq_eff.T)   # [768, 512]
    wk_t = np.ascontiguousarray(wk_eff.T)
    wv_t = np.ascontiguousarray(wv_eff.T)
    wf_t = np.ascontiguousarray(wfin.T)     # [512, 256]
    bq_c = np.ascontiguousarray(bq_eff.reshape(HCH, P).T)  # [128, 4]
    bk_c = np.ascontiguousarray(bk_eff.reshape(HCH, P).T)
    bv_r = np.ascontiguousarray(bv_eff.reshape(1, HIDDEN_DIM))
    with_v_bias = bool(np.any(bv_eff != 0))

    ci = np.asarray(cell_idx).astype(np.int32)             # [2048, 64]
    ln = np.maximum(np.asarray(cell_len).astype(np.int64), 1)
    ln = np.minimum(ln, MAX_LEN).astype(np.int32)          # [2048]
    pos = np.arange(MAX_LEN, dtype=np.int32)
    valid = pos[None, :] < ln[:, None]                     # [2048, 64]
    maskb_full = np.where(valid, f32(0.0), f32(-1e30))     # [2048, 64]
    u_full = (valid / ln[:, None]).astype(f32)             # [2048, 64]

    in_maps = []
    for core in range(N_CORES):
        cs = slice(core * CELLS_PER_CORE, (core + 1) * CELLS_PER_CORE)
        idx_c = np.ascontiguousarray(ci[cs].reshape(-1))
        mb_c = np.ascontiguousarray(maskb_full[cs].reshape(-1))
        u_c = u_full[cs]                                   # [256, 64]
        u2_c = np.zeros((TILES_PER_CORE, P, 2), f32)
        u2_c[:, 0:64, 0] = u_c[0::2]
        u2_c[:, 64:128, 1] = u_c[1::2]
        in_maps.append({
            "table": cf,
            "wq_t": wq_t, "wk_t": wk_t, "wv_t": wv_t, "wf_t": wf_t,
            "bq_c": bq_c, "bk_c": bk_c, "bv_r": bv_r,
            "idx": idx_c, "maskb": mb_c,
            "u2": u2_c.reshape(TILES_PER_CORE * P, 2),
        })
    return in_maps, b_final, with_v_bias


_NC_CACHE: dict = {}


def get_nc(with_v_bias: bool):
    if with_v_bias not in _NC_CACHE:
        _NC_CACHE[with_v_bias] = build_kernel(with_v_bias)
    return _NC_CACHE[with_v_bias]


def kernel(**inputs) -> np.ndarray:
    in_maps, b_final, with_v_bias = preprocess(**inputs)
    nc = get_nc(with_v_bias)
    res = run_bass_kernel_spmd(nc, in_maps, list(range(N_CORES)))
    out = np.concatenate([res.results[i]["out"] for i in range(N_CORES)], axis=0)
    return (out + b_final[None, :]).astype(np.float32)



# revision 17
# speedup vs baseline: 1.3324x; 1.3324x over previous
"""Trainium2 Bass kernel for nn_AttentionCellEncoder (optimized).

Contract: kernel(**inputs) takes FULL unsharded inputs (as produced by
setup_inputs) and returns the FULL [2048, 256] float32 output. Internally
shards cells across 8 NeuronCores, runs a Bass/Tile kernel via
run_bass_kernel_spmd, and reassembles the output.

Strategy vs the straightforward version:
  * bf16 everywhere on device (tolerance 2e-2; measured end-to-end err ~4e-3).
    Attention matmuls have small free dims where fp32 runs at 1/4 rate.
  * Ragged-aware packing: cells are bin-packed by true length into 128-token
    tiles (up to CMAX cells per tile) instead of one fixed 64-token slot per
    cell; with uniform lengths this drops ~45% of all gather/matmul work.
  * Full-tile scores + multiplicative 0/1 block-diagonal mask, so attention
    uses 128-deep contractions and few large matmuls.
  * Per-input runtime specialization: the kernel is traced/compiled for the
    actual packing of the given cell_len distribution.

Self-contained: all shapes hardcoded; no file I/O.
"""

import numpy as np
import ml_dtypes

import concourse.bass as bass
import concourse.mybir as mybir
import concourse.tile as tile
from concourse import bacc
from concourse.bass_utils import run_bass_kernel_spmd
from concourse.masks import make_identity

FP = mybir.dt.float32
BF = mybir.dt.bfloat16
I32 = mybir.dt.int32
NPBF = ml_dtypes.bfloat16
P = 128

# Problem dims
NUM_HEADS = 8
NUM_CHUNKS, INPUT_DIM = 50000, 768   # D = 768
HIDDEN_DIM, OUTPUT_DIM = 512, 256    # H = 512
NUM_CELLS, MAX_LEN = 2048, 64        # C, L
HEAD_DIM = HIDDEN_DIM // NUM_HEADS   # 64

N_CORES = 8
CMAX = 16                 # max cells packed into one 128-token tile
DCH = INPUT_DIM // P      # 6 d-chunks
HCH = HIDDEN_DIM // P     # 4 h-chunks
TPB = 4                   # tiles per block (512-token QKV blocks)
# wts column offsets
WQ0, WK0, WV0, WF0 = 0, DCH * HIDDEN_DIM, 2 * DCH * HIDDEN_DIM, 3 * DCH * HIDDEN_DIM
WCOLS = 3 * DCH * HIDDEN_DIM + HCH * OUTPUT_DIM  # 9216 + 1024


def build_kernel(T: int, with_q_bias: bool, with_v_bias: bool, repeat: int = 1,
                 stage: int = 99):
    """Trace + compile the per-core SPMD kernel for T tiles/core.

    stage: truncate the per-block body for HW bisection (1=gather+transpose,
    2=+qkv, 3=+scores/exp/mask, 4=+ctx/normalize, 99=full)."""
    assert T % 8 == 0
    nc = bacc.Bacc(None)

    table = nc.dram_tensor("table", [NUM_CHUNKS, INPUT_DIM], BF, kind="ExternalInput")
    wts = nc.dram_tensor("wts", [P, WCOLS], BF, kind="ExternalInput")
    idxs = nc.dram_tensor("idxs", [P, T], I32, kind="ExternalInput")
    bmask = nc.dram_tensor("bmask", [T * P, P], BF, kind="ExternalInput")
    uw = nc.dram_tensor("uw", [T * P, CMAX], BF, kind="ExternalInput")
    if with_q_bias:
        bq_c = nc.dram_tensor("bq_c", [P, HCH], FP, kind="ExternalInput")
    if with_v_bias:
        bv_r = nc.dram_tensor("bv_r", [1, HIDDEN_DIM], BF, kind="ExternalInput")
    out = nc.dram_tensor("out", [T * CMAX, OUTPUT_DIM], FP, kind="ExternalOutput")

    with tile.TileContext(nc) as tc:
        with (
            tc.tile_pool(name="const", bufs=1) as cpool,
            tc.tile_pool(name="xp", bufs=3) as xpool,
            tc.tile_pool(name="blk", bufs=2) as bpool,
            tc.tile_pool(name="sm", bufs=2) as spool,
            tc.tile_pool(name="op", bufs=2) as opool,
            tc.tile_pool(name="ps", bufs=2, space="PSUM") as pspool,
        ):
            ident = cpool.tile([P, P], BF)
            make_identity(nc, ident[:])
            ones = cpool.tile([P, 1], BF)
            nc.gpsimd.memset(ones[:], 1.0)
            wsb = cpool.tile([P, WCOLS], BF)
            nc.sync.dma_start(out=wsb[:], in_=wts[:, :])
            idx_sb = cpool.tile([P, T], I32)
            nc.sync.dma_start(out=idx_sb[:], in_=idxs[:, :])
            poolsb = cpool.tile([P, T * HCH * CMAX], BF)
            if stage < 99:
                nc.gpsimd.memset(poolsb[:], 0.0)
            if with_q_bias:
                bq_sb = cpool.tile([P, HCH], FP)
                nc.sync.dma_start(out=bq_sb[:], in_=bq_c[:, :])
            if with_v_bias:
                ones1 = cpool.tile([1, P], BF)
                nc.gpsimd.memset(ones1[:], 1.0)
                bv_sb = cpool.tile([1, HIDDEN_DIM], BF)
                nc.sync.dma_start(out=bv_sb[:], in_=bv_r[:, :])

            def gather_transpose(b):
                """Indirect-gather 4 tiles of block b and transpose to d-major."""
                xT = bpool.tile([P, DCH * TPB * P], BF, tag="xT")
                for t4 in range(TPB):
                    t = b * TPB + t4
                    x = xpool.tile([P, INPUT_DIM], BF, tag="x")
                    nc.gpsimd.indirect_dma_start(
                        out=x[:], out_offset=None, in_=table[:],
                        in_offset=bass.IndirectOffsetOnAxis(
                            ap=idx_sb[:, t:t + 1], axis=0),
                    )
                    for half in range(2):
                        pa = pspool.tile([P, 3 * P], BF, tag="xp")
                        for j in range(3):
                            jj = half * 3 + j
                            nc.tensor.transpose(
                                out=pa[:, j * P:(j + 1) * P],
                                in_=x[:, jj * P:(jj + 1) * P],
                                identity=ident[:])
                        nc.vector.tensor_copy(
                            out=xT[:].rearrange("p (j n) -> p j n", j=DCH)
                                [:, half * 3:half * 3 + 3, t4 * P:(t4 + 1) * P],
                            in_=pa[:].rearrange("p (j n) -> p j n", j=3),
                        )
                return xT

            def qk_proj(xT):
                """qT/kT [128 = 2 heads x 64 hd, hc*512 + tok] + odd-head
                copies at partition base 0 (mixed-quadrant matmuls abort on
                HW, so scores always use base-0 operands)."""
                qT = bpool.tile([P, HCH * TPB * P], BF, tag="qT")
                kT = bpool.tile([P, HCH * TPB * P], BF, tag="kT")
                qTo = bpool.tile([64, HCH * TPB * P], BF, tag="qTo")
                kTo = bpool.tile([64, HCH * TPB * P], BF, tag="kTo")
                for (w0, dst, on_act) in ((WQ0, qT, True), (WK0, kT, False)):
                    for hc in range(HCH):
                        acc = pspool.tile([P, TPB * P], FP, tag="acc")
                        for j in range(DCH):
                            nc.tensor.matmul(
                                out=acc[:],
                                lhsT=wsb[:, w0 + j * HIDDEN_DIM + hc * P:
                                         w0 + j * HIDDEN_DIM + (hc + 1) * P],
                                rhs=xT[:, j * TPB * P:(j + 1) * TPB * P],
                                start=(j == 0), stop=(j == DCH - 1),
                            )
                        d = dst[:, hc * TPB * P:(hc + 1) * TPB * P]
                        if on_act:
                            if with_q_bias:
                                nc.scalar.activation(
                                    out=d, in_=acc[:],
                                    func=mybir.ActivationFunctionType.Identity,
                                    bias=bq_sb[:, hc:hc + 1])
                            else:
                                nc.scalar.activation(
                                    out=d, in_=acc[:],
                                    func=mybir.ActivationFunctionType.Copy)
                        else:
                            nc.vector.tensor_copy(out=d, in_=acc[:])
                nc.sync.dma_start(out=qTo[0:64, :], in_=qT[64:P, :])
                nc.sync.dma_start(out=kTo[0:64, :], in_=kT[64:P, :])
                return qT, kT, qTo, kTo

            def v_tile(xT, v, t4):
                """v[:, t4*512:(t4+1)*512] = x_tile @ Wv (+bias)."""
                acc = pspool.tile([P, HIDDEN_DIM], FP, tag="acc")
                nmm = DCH + (1 if with_v_bias else 0)
                for j in range(DCH):
                    nc.tensor.matmul(
                        out=acc[:],
                        lhsT=xT[:, j * TPB * P + t4 * P:
                                j * TPB * P + (t4 + 1) * P],
                        rhs=wsb[:, WV0 + j * HIDDEN_DIM:
                                WV0 + (j + 1) * HIDDEN_DIM],
                        start=(j == 0), stop=(j == nmm - 1),
                    )
                if with_v_bias:
                    nc.tensor.matmul(out=acc[:], lhsT=ones1[0:1, :],
                                     rhs=bv_sb[0:1, :], start=False, stop=True)
                nc.scalar.activation(
                    out=v[:, t4 * HIDDEN_DIM:(t4 + 1) * HIDDEN_DIM],
                    in_=acc[:], func=mybir.ActivationFunctionType.Copy)

            def att1(st):
                """scores -> exp -> 0/1-mask for block st['b']; fills
                st['ems'], st['us']."""
                b, qT, kT, qTo, kTo = st["b"], st["qT"], st["kT"], st["qTo"], st["kTo"]
                for t4 in range(TPB):
                    t = b * TPB + t4
                    B = spool.tile([P, P], BF, tag="B")
                    nc.sync.dma_start(out=B[:], in_=bmask[t * P:(t + 1) * P, :])
                    u_sb = spool.tile([P, CMAX], BF, tag="u", bufs=5)
                    nc.sync.dma_start(out=u_sb[:], in_=uw[t * P:(t + 1) * P, :])
                    st["us"].append(u_sb)
                    e = spool.tile([P, NUM_HEADS * P], BF, tag="e")
                    for half in range(2):
                        sc = pspool.tile([P, 4 * P], FP, tag="sc")
                        for hh in range(4):
                            h = half * 4 + hh
                            kk, qq = (kT, qT) if h % 2 == 0 else (kTo, qTo)
                            col = (h // 2) * TPB * P + t4 * P
                            nc.tensor.matmul(
                                out=sc[:, hh * P:(hh + 1) * P],
                                lhsT=kk[0:64, col:col + P],
                                rhs=qq[0:64, col:col + P],
                                start=True, stop=True,
                            )
                        nc.scalar.activation(
                            out=e[:, half * 4 * P:(half + 1) * 4 * P],
                            in_=sc[:],
                            func=mybir.ActivationFunctionType.Exp)
                    em = spool.tile([P, NUM_HEADS * P], BF, tag="em", bufs=5)
                    nc.vector.tensor_tensor(
                        out=em[:].rearrange("p (h l) -> p h l", h=NUM_HEADS),
                        in0=e[:].rearrange("p (h l) -> p h l", h=NUM_HEADS),
                        in1=B[:, None, :].to_broadcast([P, NUM_HEADS, P]),
                        op=mybir.AluOpType.mult,
                    )
                    st["ems"].append(em)

            def att2_tile(st, t4):
                """ctx/den -> normalize -> pool for tile t4 of block st['b']."""
                b, v, em, u_sb = st["b"], st["v"], st["ems"][t4], st["us"][t4]
                t = b * TPB + t4
                cd = pspool.tile([P, HIDDEN_DIM + NUM_HEADS], FP, tag="cd",
                                 bufs=1)
                for h in range(NUM_HEADS):
                    el = em[:, h * P:(h + 1) * P]
                    nc.tensor.matmul(
                        out=cd[:, h * 64:(h + 1) * 64],
                        lhsT=el,
                        rhs=v[:, t4 * HIDDEN_DIM + h * 64:
                              t4 * HIDDEN_DIM + (h + 1) * 64],
                        start=True, stop=True,
                    )
                    nc.tensor.matmul(
                        out=cd[:, HIDDEN_DIM + h:HIDDEN_DIM + h + 1],
                        lhsT=el, rhs=ones[:, 0:1],
                        start=True, stop=True,
                    )
                r = spool.tile([P, NUM_HEADS], FP, tag="r")
                nc.vector.reciprocal(
                    out=r[:], in_=cd[:, HIDDEN_DIM:HIDDEN_DIM + NUM_HEADS])
                cn = spool.tile([P, HIDDEN_DIM], BF, tag="cn")
                nc.vector.tensor_tensor(
                    out=cn[:].rearrange("p (h d) -> p h d", h=NUM_HEADS),
                    in0=cd[:, 0:HIDDEN_DIM]
                        .rearrange("p (h d) -> p h d", h=NUM_HEADS),
                    in1=r[:, :, None].to_broadcast([P, NUM_HEADS, HEAD_DIM]),
                    op=mybir.AluOpType.mult,
                )
                pt = pspool.tile([P, HCH * CMAX], FP, tag="xp")
                for hc in range(HCH):
                    nc.tensor.matmul(
                        out=pt[:, hc * CMAX:(hc + 1) * CMAX],
                        lhsT=cn[:, hc * P:(hc + 1) * P],
                        rhs=u_sb[:],
                        start=True, stop=True,
                    )
                # poolsb layout: [p, g, hc, slot] with slot = tl*16+j
                g, tl = t // 8, t % 8
                dst = poolsb[:, g * 8 * HCH * CMAX:(g + 1) * 8 * HCH * CMAX]
                dst = dst.rearrange("p (h s) -> p h s", h=HCH)
                nc.vector.tensor_copy(
                    out=dst[:, :, tl * CMAX:(tl + 1) * CMAX],
                    in_=pt[:].rearrange("p (h j) -> p h j", h=HCH))

            NB = T // TPB
            for _rep in range(repeat):
                # one-block software pipeline: attention of block i-1 overlaps
                # the gather/transpose/projections of block i, so the PE never
                # waits on the exp/mask round-trip through ACT/DVE.
                prev = None
                for i in range(NB + 1):
                    xT = gather_transpose(i) if i < NB else None
                    if prev is not None:
                        att1(prev)
                    if i < NB:
                        qT, kT, qTo, kTo = qk_proj(xT)
                        v = bpool.tile([P, TPB * HIDDEN_DIM], BF, tag="v")
                    for t4 in range(TPB):
                        if prev is not None:
                            att2_tile(prev, t4)
                        if i < NB:
                            v_tile(xT, v, t4)
                    if i < NB:
                        prev = {"b": i, "qT": qT, "kT": kT, "qTo": qTo,
                                "kTo": kTo, "v": v, "ems": [], "us": []}

                # ---- final projection per group of 8 tiles (128 cell slots) ----
                for g in range(T // 8):
                    acc = pspool.tile([P, OUTPUT_DIM], FP, tag="acc")
                    pg0 = g * 8 * HCH * CMAX
                    for hc in range(HCH):
                        nc.tensor.matmul(
                            out=acc[:], lhsT=poolsb[:, pg0 + hc * P:pg0 + (hc + 1) * P],
                            rhs=wsb[:, WF0 + hc * OUTPUT_DIM:
                                    WF0 + (hc + 1) * OUTPUT_DIM],
                            start=(hc == 0), stop=(hc == HCH - 1),
                        )
                    osb = opool.tile([P, OUTPUT_DIM], FP, tag="osb")
                    nc.scalar.activation(out=osb[:], in_=acc[:],
                                         func=mybir.ActivationFunctionType.Copy)
                    nc.sync.dma_start(out=out[g * P:(g + 1) * P, :], in_=osb[:])

    nc.compile()
    return nc


def pack_cells(lens: np.ndarray):
    """Assign cells to cores and bin-pack each core's cells into 128-token
    tiles (<= CMAX cells/tile). Returns (packs, T): packs[core] = list of
    bins, each bin a list of cell ids; T = uniform tile count per core."""
    order = np.argsort(-lens, kind="stable")
    core_tokens = np.zeros(N_CORES, np.int64)
    core_cells: list[list[int]] = [[] for _ in range(N_CORES)]
    for c in order:
        k = int(np.argmin(core_tokens))
        core_cells[k].append(int(c))
        core_tokens[k] += lens[c]
    packs = []
    for k in range(N_CORES):
        bins: list[list] = []   # [remaining, count, cells]
        for c in core_cells[k]:  # desc length order
            L = int(lens[c])
            for bn in bins:
                if bn[0] >= L and bn[1] < CMAX:
                    bn[0] -= L
                    bn[1] += 1
                    bn[2].append(c)
                    break
            else:
                bins.append([P - L, 1, [c]])
        packs.append([bn[2] for bn in bins])
    T = max(len(p) for p in packs)
    T = ((T + 7) // 8) * 8
    return packs, T


def preprocess(chunk_features, Wq, bq, Wk, bk, Wv, bv, W_in, b_in, Wo, bo,
               Wout, bout, cell_idx, cell_len):
    """Host-side weight folding, cell packing, per-core input maps.

    Returns (in_maps, b_final, slot_of_cell [2048] -> (core, row), T,
    with_q_bias, with_v_bias)."""
    f32 = np.float32
    cf = np.asarray(chunk_features, f32)
    Wq, Wk, Wv = (np.asarray(w, f32) for w in (Wq, Wk, Wv))
    bq, bk, bv = (np.asarray(x, f32) for x in (bq, bk, bv))
    W_in = np.asarray(W_in, f32)
    b_in = np.asarray(b_in, f32)
    Wo, bo = np.asarray(Wo, f32), np.asarray(bo, f32)
    Wout, bout = np.asarray(Wout, f32), np.asarray(bout, f32)

    Wiq, Wik, Wiv = np.split(W_in, 3, axis=0)
    biq, bik, biv = np.split(b_in, 3)
    scale = f32(1.0 / np.sqrt(HEAD_DIM))
    wq_eff = (Wiq @ Wq) * scale          # [512, 768]
    wk_eff = Wik @ Wk
    wv_eff = Wiv @ Wv
    bq_eff = (Wiq @ bq + biq) * scale    # [512]; k-bias is softmax-invariant
    bv_eff = Wiv @ bv + biv
    wfin = Wout @ Wo                     # [256, 512]
    b_final = bo @ Wout.T + bout         # [256]
    with_q_bias = bool(np.any(bq_eff != 0))
    with_v_bias = bool(np.any(bv_eff != 0))

    # wts packing: [128, WCOLS] bf16; w*_sb[p, j*512 + h] = w_eff.T[j*128+p, h]
    wts = np.zeros((P, WCOLS), NPBF)
    for w0, w_eff in ((WQ0, wq_eff), (WK0, wk_eff), (WV0, wv_eff)):
        wt = np.ascontiguousarray(w_eff.T)          # [768, 512]
        for j in range(DCH):
            wts[:, w0 + j * HIDDEN_DIM:w0 + (j + 1) * HIDDEN_DIM] = \
                wt[j * P:(j + 1) * P, :].astype(NPBF)
    wft = np.ascontiguousarray(wfin.T)              # [512, 256]
    for hc in range(HCH):
        wts[:, WF0 + hc * OUTPUT_DIM:WF0 + (hc + 1) * OUTPUT_DIM] = \
            wft[hc * P:(hc + 1) * P, :].astype(NPBF)

    table_b = cf.astype(NPBF)
    ci = np.asarray(cell_idx).astype(np.int32)             # [2048, 64]
    ln = np.maximum(np.asarray(cell_len).astype(np.int64), 1)
    ln = np.minimum(ln, MAX_LEN).astype(np.int32)          # [2048]

    packs, T = pack_cells(ln)

    slot_core = np.zeros(NUM_CELLS, np.int32)
    slot_row = np.zeros(NUM_CELLS, np.int32)
    in_maps = []
    for core in range(N_CORES):
        bins = packs[core]
        idxs = np.zeros((P, T), np.int32)
        bm = np.zeros((T, P, P), NPBF)
        u = np.zeros((T, P, CMAX), NPBF)
        for t in range(T):
            pos = 0
            if t < len(bins):
                for j, c in enumerate(bins[t]):
                    L = int(ln[c])
                    idxs[pos:pos + L, t] = ci[c, :L]
                    bm[t, pos:pos + L, pos:pos + L] = NPBF(1.0)
                    u[t, pos:pos + L, j] = NPBF(1.0 / L)
                    slot_core[c] = core
                    slot_row[c] = t * CMAX + j
                    pos += L
            # padding slots: self-attend so the softmax denominator stays > 0
            for l in range(pos, P):
                bm[t, l, l] = NPBF(1.0)
        m = {
            "table": table_b, "wts": wts, "idxs": idxs,
            "bmask": bm.reshape(T * P, P), "uw": u.reshape(T * P, CMAX),
        }
        if with_q_bias:
            m["bq_c"] = np.ascontiguousarray(bq_eff.reshape(HCH, P).T)
        if with_v_bias:
            m["bv_r"] = bv_eff.reshape(1, HIDDEN_DIM).astype(NPBF)
        in_maps.append(m)
    return in_maps, b_final, (slot_core, slot_row), T, with_q_bias, with_v_bias


_NC_CACHE: dict = {}


def get_nc(T: int, with_q_bias: bool, with_v_bias: bool):
    key = (T, with_q_bias, with_v_bias)
    if key not in _NC_CACHE:
        _NC_CACHE[key] = build_kernel(T, with_q_bias, with_v_bias)
    return _NC_CACHE[key]


def kernel(**inputs) -> np.ndarray:
    in_maps, b_final, (slot_core, slot_row), T, wqb, wvb = preprocess(**inputs)
    nc = get_nc(T, wqb, wvb)
    res = run_bass_kernel_spmd(nc, in_maps, list(range(N_CORES)))
    outs = [np.asarray(res.results[i]["out"]) for i in range(N_CORES)]
    full = np.empty((NUM_CELLS, OUTPUT_DIM), np.float32)
    for c in range(NUM_CELLS):
        full[c] = outs[slot_core[c]][slot_row[c]]
    return (full + b_final[None, :]).astype(np.float32)
